# revision 3
# baseline (speedup 1.0000x reference)
"""Trainium2 Bass kernel for CrossAttention3D — fp8e4 DoubleRow version.

Math (per batch b, x = q[b] viewed (C, S)):
    qp = Wq x + bq ; kp = Wk x + bk ; vf = x^T Wv^T + bv
    simT = kp^T qp * C^-0.5 ; E = exp(simT) ; den = colsum(E)
    out = Wo (vf^T E) / den + bo

Q/K/V projections and attention matmuls run as fp8e4 DoubleRow
(contraction 256/instr, 2x bf16 PE throughput). Static scales keep every
fp8 operand in e4m3's good range (max 240, min normal 2^-6):
    wq8/wk8 = 256*W.T (wk also carries an fp8 residual term for
    accuracy), wv8 = 64*Wv.T, x8 = x (sigma~1); qp8/kp8 true-scale
    (sigma~0.45); et = exp(C^-0.5 * sim) in [0.2, 4.6]; vf8 = 64*v.
xu (= E^T v) is evicted in bf16 and the output projection runs in bf16
to keep the final-path quantization error small. out = po/den + bo.

Sharding: 8 cores = 4 batches x 2 query-halves. All inputs are
pre-arranged host-side into SBUF layout so DMA lines are contiguous.
Engines: PE matmuls (incl. ones-colsum den); ACT kp/qp/xu evict + exp;
DVE vf evict + final normalize; GPSIMD broadcasts 1/den on-chip;
normalization applied after the (linear) output projection so the
reciprocal chain overlaps the O-proj matmuls.
"""

import numpy as np
import ml_dtypes

import concourse.bass as bass
import concourse.bacc as bacc
import concourse.tile as tile
from concourse import mybir
from concourse.bass_utils import run_bass_kernel_spmd

AF = mybir.ActivationFunctionType
F32 = mybir.dt.float32
BF16 = mybir.dt.bfloat16
FP8 = mybir.dt.float8e4
DRM = mybir.MatmulPerfMode.DoubleRow

B, C, H, W = 4, 512, 64, 64
S = H * W            # 4096 tokens
P = 128              # partitions
CC = C // P          # 4 channel chunks
QH = S // 2          # 2048 queries per core
FB = 512             # free-dim block
NQB = QH // FB       # 4 query blocks per core
NSB = S // FB        # 8 token blocks
NKC = S // P         # 32 key chunks
NKP = NKC // 2       # 16 key chunk pairs
N_CORES = 8

SW = 256.0           # weight scale for Wq/Wk/Wo
SV = 64.0            # weight scale for Wv
SCALE = float(C) ** -0.5


def _build_bass() -> bass.Bass:
    nc = bacc.Bacc("TRN2", target_bir_lowering=False)

    # all inputs pre-arranged host-side into SBUF layout (contiguous
    # per-partition DMA lines): x/xq [p, blk, cc, 512]
    x_d = nc.dram_tensor("x", [P, NSB, CC, FB], FP8, kind="ExternalInput")
    xq_d = nc.dram_tensor("xq", [P, NQB, CC, FB], FP8, kind="ExternalInput")
    # packed fp8 weights [p, w, o]: w1 = [wv|wk|wkr], w2 = [wq|wqr]
    w1_d = nc.dram_tensor("w1", [P, 3 * CC, C], FP8, kind="ExternalInput")
    w2_d = nc.dram_tensor("w2", [P, CC, C], FP8, kind="ExternalInput")
    wo_d = nc.dram_tensor("wo", [P, CC, C], BF16, kind="ExternalInput")  # Wo.T
    bq_d = nc.dram_tensor("bq", [C], F32, kind="ExternalInput")
    bk_d = nc.dram_tensor("bk", [C], F32, kind="ExternalInput")
    bv_d = nc.dram_tensor("bv", [C], F32, kind="ExternalInput")     # bv * SV
    bo_d = nc.dram_tensor("bo", [C], F32, kind="ExternalInput")
    out_d = nc.dram_tensor("out", [C, QH], BF16, kind="ExternalOutput")

    out_r = out_d[:, :].rearrange("(oc p) s -> p oc s", p=P)

    def bcast_ap(ap_1d, parts):
        # [n] -> [parts, n] via 0-stride partition dim (DMA-only pattern)
        return bass.AP(
            tensor=ap_1d.tensor, offset=ap_1d.offset,
            ap=[[0, parts]] + [list(d) for d in ap_1d.ap],
        )

    with tile.TileContext(nc) as tc:
        with (
            tc.tile_pool(name="consts", bufs=1) as consts,
            tc.tile_pool(name="kv", bufs=1) as kv,
            tc.tile_pool(name="xs", bufs=3) as xs,
            tc.tile_pool(name="qps", bufs=2) as qps,
            tc.tile_pool(name="ets", bufs=3) as ets,
            tc.tile_pool(name="xus", bufs=2) as xus,
            tc.tile_pool(name="outs", bufs=4) as outs,
            tc.tile_pool(name="dscr", bufs=2, space="DRAM") as dscr,
            tc.tile_pool(name="psmm", bufs=3, space="PSUM") as psmm,
            tc.tile_pool(name="psav", bufs=4, space="PSUM") as psav,
            tc.tile_pool(name="psden", bufs=1, space="PSUM") as psden,
        ):
            # ---- constants; first x block + phase-1 weights lead so PE
            # ---- starts as early as possible, Wq/Wo deferred to phase 2 ----
            xt_first = xs.tile([P, CC, FB], FP8, tag="xt", name="xt_first")
            nc.sync.dma_start(out=xt_first, in_=x_d[:, 0, :, :])
            # weights on the scalar queue (parallel to sync queue): wv alone
            # first so the first V matmul starts ASAP, then wk+wkr
            w1_sb = consts.tile([P, 3 * CC, C], FP8, tag="w1")
            nc.scalar.dma_start(out=w1_sb[:, 0:CC, :], in_=w1_d[:, 0:CC, :])
            nc.scalar.dma_start(out=w1_sb[:, CC:3 * CC, :], in_=w1_d[:, CC:3 * CC, :])
            wv_sb = w1_sb[:, 0:CC, :]
            wk_sb = w1_sb[:, CC:2 * CC, :]
            wkr_sb = w1_sb[:, 2 * CC:3 * CC, :]
            bvb_sb = consts.tile([P, C], F32, tag="bvb")
            nc.sync.dma_start(out=bvb_sb, in_=bcast_ap(bv_d[:], P))
            bk_sb = consts.tile([P, CC], F32, tag="bk")
            nc.sync.dma_start(out=bk_sb, in_=bk_d[:].rearrange("(cc p) -> p cc", p=P))
            ones_col = consts.tile([P, 2, 16], FP8, tag="ones_col")
            nc.vector.memset(ones_col, 1.0)

            # ---- persistent K / V ----
            kp_sb = kv.tile([P, CC, S], FP8, tag="kp")
            vf_sb = kv.tile([P, NKC, C], FP8, tag="vf")

            # ---- phase 1: K and V projections over the full sequence ----
            for sb in range(NSB):
                if sb == 0:
                    xt = xt_first
                else:
                    xt = xs.tile([P, CC, FB], FP8, tag="xt")
                    nc.sync.dma_start(out=xt, in_=x_d[:, sb, :, :])
                for i4 in range(CC):
                    sc = sb * CC + i4
                    psv = psav.tile([P, FB], F32, tag="av")
                    for h in range(2):
                        nc.tensor.matmul(
                            psv, lhsT=xt[:, 2 * h:2 * h + 2, i4 * P:(i4 + 1) * P],
                            rhs=wv_sb[:, 2 * h:2 * h + 2, :],
                            start=(h == 0), stop=(h == 1), perf_mode=DRM,
                        )
                    # vf = 64*v + 64*bv  (bias pre-scaled host-side)
                    nc.vector.tensor_add(out=vf_sb[:, sc, :], in0=psv[:], in1=bvb_sb[:])
                for oc in range(CC):
                    ps = psmm.tile([P, FB], F32, tag="mm")
                    for wt, h in ((w, h) for w in (wk_sb, wkr_sb) for h in range(2)):
                        nc.tensor.matmul(
                            ps, lhsT=wt[:, 2 * h:2 * h + 2, oc * P:(oc + 1) * P],
                            rhs=xt[:, 2 * h:2 * h + 2, :],
                            start=(wt is wk_sb and h == 0),
                            stop=(wt is wkr_sb and h == 1), perf_mode=DRM,
                        )
                    nc.scalar.activation(
                        out=kp_sb[:, oc, sb * FB:(sb + 1) * FB], in_=ps[:],
                        func=AF.Identity, bias=bk_sb[:, oc:oc + 1], scale=1.0 / SW,
                    )

            # ---- deferred constants for phase 2 ----
            w2_sb = consts.tile([P, CC, C], FP8, tag="w2")
            nc.scalar.dma_start(out=w2_sb, in_=w2_d[:, :, :])
            wq_sb = w2_sb[:, 0:CC, :]
            wo_sb = consts.tile([P, CC, C], BF16, tag="wo")
            nc.scalar.dma_start(out=wo_sb, in_=wo_d[:, :, :])
            bq_sb = consts.tile([P, CC], F32, tag="bq")
            nc.sync.dma_start(out=bq_sb, in_=bq_d[:].rearrange("(cc p) -> p cc", p=P))
            bo_sb = consts.tile([P, CC], F32, tag="bo")
            nc.sync.dma_start(out=bo_sb, in_=bo_d[:].rearrange("(cc p) -> p cc", p=P))

            # ---- phase 2: per query-block fused attention ----
            for qb in range(NQB):
                qsl = slice(qb * FB, (qb + 1) * FB)

                # Q projection
                xqt = xs.tile([P, CC, FB], FP8, tag="xt")
                nc.sync.dma_start(out=xqt, in_=xq_d[:, qb, :, :])
                qp = qps.tile([P, CC, FB], FP8, tag="qp")
                for oc in range(CC):
                    ps = psmm.tile([P, FB], F32, tag="mm")
                    for h in range(2):
                        nc.tensor.matmul(
                            ps, lhsT=wq_sb[:, 2 * h:2 * h + 2, oc * P:(oc + 1) * P],
                            rhs=xqt[:, 2 * h:2 * h + 2, :],
                            start=(h == 0), stop=(h == 1), perf_mode=DRM,
                        )
                    nc.scalar.activation(
                        out=qp[:, oc, :], in_=ps[:],
                        func=AF.Identity, bias=bq_sb[:, oc:oc + 1], scale=1.0 / SW,
                    )

                den = psden.tile([16, FB], F32, tag="den")
                avt = [psav.tile([P, FB], F32, tag="av", name=f"avt{qb}_{i}") for i in range(CC)]
                for t in range(NKP):
                    et = ets.tile([P, 2, FB], FP8, tag="et")
                    for j in range(2):
                        kc = 2 * t + j
                        simt = psmm.tile([P, FB], F32, tag="mm")
                        for h in range(2):
                            nc.tensor.matmul(
                                simt,
                                lhsT=kp_sb[:, 2 * h:2 * h + 2, kc * P:(kc + 1) * P],
                                rhs=qp[:, 2 * h:2 * h + 2, :],
                                start=(h == 0), stop=(h == 1), perf_mode=DRM,
                            )
                        nc.scalar.activation(
                            out=et[:, j, :], in_=simt[:], func=AF.Exp, scale=SCALE,
                        )
                    nc.tensor.matmul(
                        den, lhsT=ones_col[:, :, :], rhs=et[:, :, :],
                        start=(t == 0), stop=(t == NKP - 1), perf_mode=DRM,
                    )
                    for c4 in range(CC):
                        nc.tensor.matmul(
                            avt[c4],
                            lhsT=vf_sb[:, 2 * t:2 * t + 2, c4 * P:(c4 + 1) * P],
                            rhs=et[:, :, :],
                            start=(t == 0), stop=(t == NKP - 1), perf_mode=DRM,
                        )

                # 1/den, broadcast on-chip (GPSIMD) to all partitions;
                # runs concurrently with the output-projection matmuls below.
                den_sb = xs.tile([1, FB], F32, tag="den_sb")
                nc.scalar.activation(out=den_sb, in_=den[0:1, :], func=AF.Copy)
                rec = xs.tile([1, FB], F32, tag="rec")
                nc.vector.reciprocal(out=rec, in_=den_sb[:])
                rbc = xs.tile([P, FB], F32, tag="rbc")
                nc.gpsimd.partition_broadcast(rbc[:, :], rec[:, :])

                # evict unnormalized attention output: xu = avt/64 = E^T v (bf16)
                xu = xus.tile([P, CC, FB], BF16, tag="xu")
                for c4 in range(CC):
                    nc.scalar.activation(
                        out=xu[:, c4, :], in_=avt[c4][:], func=AF.Copy, scale=1.0 / SV,
                    )

                for oc in range(CC):
                    po = psav.tile([P, FB], F32, tag="av", name=f"po{qb}_{oc}")
                    for cc in range(CC):
                        nc.tensor.matmul(
                            po, lhsT=wo_sb[:, cc, oc * P:(oc + 1) * P],
                            rhs=xu[:, cc, :],
                            start=(cc == 0), stop=(cc == CC - 1),
                        )
                    # out = po/den + bo
                    ot = outs.tile([P, FB], BF16, tag="ot")
                    nc.vector.tensor_mul(out=ot, in0=po[:], in1=rbc[:])
                    ob = outs.tile([P, FB], BF16, tag="ob")
                    nc.vector.tensor_scalar_add(out=ob, in0=ot[:], scalar1=bo_sb[:, oc:oc + 1])
                    nc.sync.dma_start(out=out_r[:, oc, qsl], in_=ob[:])

    nc.finalize()
    return nc


_NC_CACHE = {}


def _get_nc() -> bass.Bass:
    if "nc" not in _NC_CACHE:
        _NC_CACHE["nc"] = _build_bass()
    return _NC_CACHE["nc"]


def make_in_maps(q, Wq, bq, Wk, bk, Wv, bv, Wo, bo):
    f = np.float32
    f8 = ml_dtypes.float8_e4m3
    bf = ml_dtypes.bfloat16
    wq = (np.asarray(Wq, f).T * SW).astype(f8)
    wk_full = np.asarray(Wk, f).T * SW
    wk = wk_full.astype(f8)
    wkr = (wk_full - wk.astype(f)).astype(f8)
    wv = (np.asarray(Wv, f).T * SV).astype(f8)
    wo = np.asarray(Wo, f).T.astype(bf)

    def to_pwo(w):  # [C(c), C(o)] -> [p, cc, o] with c = cc*128 + p
        return w.reshape(CC, P, C).transpose(1, 0, 2)

    w1 = np.ascontiguousarray(
        np.concatenate([to_pwo(wv), to_pwo(wk), to_pwo(wkr)], axis=1))
    w2 = np.ascontiguousarray(to_pwo(wq))
    wo = np.ascontiguousarray(to_pwo(wo))
    bq = np.asarray(bq, f)
    bk = np.asarray(bk, f)
    bv = np.asarray(bv, f) * f(SV)
    bo = np.asarray(bo, f)
    in_maps = []
    for core in range(N_CORES):
        b, half = core // 2, core % 2
        x = np.asarray(q[b], f).reshape(C, S).astype(f8)
        # [C, S] -> [p, sb, cc, j]  (c = cc*128 + p, s = sb*512 + j)
        xa = np.ascontiguousarray(
            x.reshape(CC, P, NSB, FB).transpose(1, 2, 0, 3))
        xqh = x[:, half * QH:(half + 1) * QH]
        xq = np.ascontiguousarray(
            xqh.reshape(CC, P, NQB, FB).transpose(1, 2, 0, 3))
        x = xa
        in_maps.append({
            "x": np.ascontiguousarray(x), "xq": xq,
            "w1": w1, "w2": w2, "wo": wo,
            "bq": bq, "bk": bk, "bv": bv, "bo": bo,
        })
    return in_maps


def gather_out(per_core_outs):
    out = np.zeros((B, C, S), np.float32)
    for core in range(N_CORES):
        b, half = core // 2, core % 2
        out[b, :, half * QH:(half + 1) * QH] = np.asarray(per_core_outs[core]).astype(np.float32)
    return out.reshape(B, C, H, W)


def kernel(q, Wq, bq, Wk, bk, Wv, bv, Wo, bo):
    nc = _get_nc()
    in_maps = make_in_maps(q, Wq, bq, Wk, bk, Wv, bv, Wo, bo)
    res = run_bass_kernel_spmd(nc, in_maps, core_ids=list(range(N_CORES)))
    return gather_out([res.results[i]["out"] for i in range(N_CORES)])


# revision 4
# speedup vs baseline: 1.0044x; 1.0044x over previous
"""Trainium2 Bass kernel for CrossAttention3D — fp8e4 DoubleRow version.

Math (per batch b, x = q[b] viewed (C, S)):
    qp = Wq x + bq ; kp = Wk x + bk ; vf = x^T Wv^T + bv
    simT = kp^T qp * C^-0.5 ; E = exp(simT) ; den = colsum(E)
    out = Wo (vf^T E) / den + bo

Q/K/V projections and attention matmuls run as fp8e4 DoubleRow
(contraction 256/instr, 2x bf16 PE throughput). Static scales keep every
fp8 operand in e4m3's good range (max 240, min normal 2^-6):
    wq8/wk8 = 256*W.T (wk also carries an fp8 residual term for
    accuracy), wv8 = 64*Wv.T, x8 = x (sigma~1); qp8/kp8 true-scale
    (sigma~0.45); et = exp(C^-0.5 * sim) in [0.2, 4.6]; vf8 = 64*v.
xu (= E^T v) is evicted in bf16 and the output projection runs in bf16
to keep the final-path quantization error small. out = po/den + bo.

Sharding: 8 cores = 4 batches x 2 query-halves. All inputs are
pre-arranged host-side into SBUF layout so DMA lines are contiguous.
Engines: PE matmuls (incl. ones-colsum den); ACT kp/qp/xu evict + exp;
DVE vf evict + final normalize; GPSIMD broadcasts 1/den on-chip;
normalization applied after the (linear) output projection so the
reciprocal chain overlaps the O-proj matmuls.
"""

import numpy as np
import ml_dtypes

import concourse.bass as bass
import concourse.bacc as bacc
import concourse.tile as tile
from concourse import mybir
from concourse.bass_utils import run_bass_kernel_spmd

AF = mybir.ActivationFunctionType
F32 = mybir.dt.float32
BF16 = mybir.dt.bfloat16
FP8 = mybir.dt.float8e4
DRM = mybir.MatmulPerfMode.DoubleRow

B, C, H, W = 4, 512, 64, 64
S = H * W            # 4096 tokens
P = 128              # partitions
CC = C // P          # 4 channel chunks
QH = S // 2          # 2048 queries per core
FB = 512             # free-dim block
NQB = QH // FB       # 4 query blocks per core
NSB = S // FB        # 8 token blocks
NKC = S // P         # 32 key chunks
NKP = NKC // 2       # 16 key chunk pairs
N_CORES = 8

SW = 256.0           # weight scale for Wq/Wk/Wo
SV = 64.0            # weight scale for Wv
SCALE = float(C) ** -0.5


def _build_bass() -> bass.Bass:
    nc = bacc.Bacc("TRN2", target_bir_lowering=False)

    # all inputs pre-arranged host-side into SBUF layout (contiguous
    # per-partition DMA lines): x/xq [p, blk, cc, 512]
    x_d = nc.dram_tensor("x", [P, NSB, CC, FB], FP8, kind="ExternalInput")
    xq_d = nc.dram_tensor("xq", [P, NQB, CC, FB], FP8, kind="ExternalInput")
    # packed fp8 weights [p, w, o]: w1 = [wv|wk|wkr], w2 = [wq|wqr]
    w1_d = nc.dram_tensor("w1", [P, 3 * CC, C], FP8, kind="ExternalInput")
    w2_d = nc.dram_tensor("w2", [P, CC, C], FP8, kind="ExternalInput")
    wo_d = nc.dram_tensor("wo", [P, CC, C], BF16, kind="ExternalInput")  # Wo.T
    bq_d = nc.dram_tensor("bq", [C], F32, kind="ExternalInput")
    bk_d = nc.dram_tensor("bk", [C], F32, kind="ExternalInput")
    bo_d = nc.dram_tensor("bo", [C], F32, kind="ExternalInput")    # bo + Wo@bv
    out_d = nc.dram_tensor("out", [C, QH], BF16, kind="ExternalOutput")

    out_r = out_d[:, :].rearrange("(oc p) s -> p oc s", p=P)

    def bcast_ap(ap_1d, parts):
        # [n] -> [parts, n] via 0-stride partition dim (DMA-only pattern)
        return bass.AP(
            tensor=ap_1d.tensor, offset=ap_1d.offset,
            ap=[[0, parts]] + [list(d) for d in ap_1d.ap],
        )

    with tile.TileContext(nc) as tc:
        with (
            tc.tile_pool(name="consts", bufs=1) as consts,
            tc.tile_pool(name="kv", bufs=1) as kv,
            tc.tile_pool(name="xs", bufs=3) as xs,
            tc.tile_pool(name="qps", bufs=2) as qps,
            tc.tile_pool(name="ets", bufs=3) as ets,
            tc.tile_pool(name="xus", bufs=2) as xus,
            tc.tile_pool(name="outs", bufs=4) as outs,
            tc.tile_pool(name="dscr", bufs=2, space="DRAM") as dscr,
            tc.tile_pool(name="psmm", bufs=3, space="PSUM") as psmm,
            tc.tile_pool(name="psav", bufs=4, space="PSUM") as psav,
            tc.tile_pool(name="psden", bufs=1, space="PSUM") as psden,
        ):
            # ---- constants; first x block + phase-1 weights lead so PE
            # ---- starts as early as possible, Wq/Wo deferred to phase 2 ----
            xt_first = xs.tile([P, CC, FB], FP8, tag="xt", name="xt_first")
            nc.sync.dma_start(out=xt_first, in_=x_d[:, 0, :, :])
            # weights on the scalar queue (parallel to sync queue): wv alone
            # first so the first V matmul starts ASAP, then wk+wkr
            w1_sb = consts.tile([P, 3 * CC, C], FP8, tag="w1")
            nc.scalar.dma_start(out=w1_sb[:, 0:CC, :], in_=w1_d[:, 0:CC, :])
            nc.scalar.dma_start(out=w1_sb[:, CC:3 * CC, :], in_=w1_d[:, CC:3 * CC, :])
            wv_sb = w1_sb[:, 0:CC, :]
            wk_sb = w1_sb[:, CC:2 * CC, :]
            wkr_sb = w1_sb[:, 2 * CC:3 * CC, :]
            bk_sb = consts.tile([P, CC], F32, tag="bk")
            nc.sync.dma_start(out=bk_sb, in_=bk_d[:].rearrange("(cc p) -> p cc", p=P))
            ones_col = consts.tile([P, 2, 16], FP8, tag="ones_col")
            nc.vector.memset(ones_col, 1.0)

            # ---- persistent K / V ----
            kp_sb = kv.tile([P, CC, S], FP8, tag="kp")
            vf_sb = kv.tile([P, NKC, C], FP8, tag="vf")

            # ---- phase 1: K and V projections over the full sequence ----
            for sb in range(NSB):
                if sb == 0:
                    xt = xt_first
                else:
                    xt = xs.tile([P, CC, FB], FP8, tag="xt")
                    nc.sync.dma_start(out=xt, in_=x_d[:, sb, :, :])
                for i4 in range(CC):
                    sc = sb * CC + i4
                    psv = psav.tile([P, FB], F32, tag="av")
                    for h in range(2):
                        nc.tensor.matmul(
                            psv, lhsT=xt[:, 2 * h:2 * h + 2, i4 * P:(i4 + 1) * P],
                            rhs=wv_sb[:, 2 * h:2 * h + 2, :],
                            start=(h == 0), stop=(h == 1), perf_mode=DRM,
                        )
                    # vf = 64*v; bv is folded into bo via bo' = bo + Wo@bv
                    nc.vector.tensor_scalar_mul(out=vf_sb[:, sc, :], in0=psv[:], scalar1=1.0)
                for oc in range(CC):
                    ps = psmm.tile([P, FB], F32, tag="mm")
                    for wt, h in ((w, h) for w in (wk_sb, wkr_sb) for h in range(2)):
                        nc.tensor.matmul(
                            ps, lhsT=wt[:, 2 * h:2 * h + 2, oc * P:(oc + 1) * P],
                            rhs=xt[:, 2 * h:2 * h + 2, :],
                            start=(wt is wk_sb and h == 0),
                            stop=(wt is wkr_sb and h == 1), perf_mode=DRM,
                        )
                    nc.scalar.activation(
                        out=kp_sb[:, oc, sb * FB:(sb + 1) * FB], in_=ps[:],
                        func=AF.Identity, bias=bk_sb[:, oc:oc + 1], scale=1.0 / SW,
                    )

            # ---- deferred constants for phase 2 ----
            w2_sb = consts.tile([P, CC, C], FP8, tag="w2")
            nc.scalar.dma_start(out=w2_sb, in_=w2_d[:, :, :])
            wq_sb = w2_sb[:, 0:CC, :]
            wo_sb = consts.tile([P, CC, C], BF16, tag="wo")
            nc.scalar.dma_start(out=wo_sb, in_=wo_d[:, :, :])
            bq_sb = consts.tile([P, CC], F32, tag="bq")
            nc.sync.dma_start(out=bq_sb, in_=bq_d[:].rearrange("(cc p) -> p cc", p=P))
            bo_sb = consts.tile([P, CC], F32, tag="bo")
            nc.sync.dma_start(out=bo_sb, in_=bo_d[:].rearrange("(cc p) -> p cc", p=P))

            # ---- phase 2: per query-block fused attention ----
            for qb in range(NQB):
                qsl = slice(qb * FB, (qb + 1) * FB)

                # Q projection
                xqt = xs.tile([P, CC, FB], FP8, tag="xt")
                nc.sync.dma_start(out=xqt, in_=xq_d[:, qb, :, :])
                qp = qps.tile([P, CC, FB], FP8, tag="qp")
                for oc in range(CC):
                    ps = psmm.tile([P, FB], F32, tag="mm")
                    for h in range(2):
                        nc.tensor.matmul(
                            ps, lhsT=wq_sb[:, 2 * h:2 * h + 2, oc * P:(oc + 1) * P],
                            rhs=xqt[:, 2 * h:2 * h + 2, :],
                            start=(h == 0), stop=(h == 1), perf_mode=DRM,
                        )
                    nc.scalar.activation(
                        out=qp[:, oc, :], in_=ps[:],
                        func=AF.Identity, bias=bq_sb[:, oc:oc + 1], scale=1.0 / SW,
                    )

                den = psden.tile([16, FB], F32, tag="den")
                avt = [psav.tile([P, FB], F32, tag="av", name=f"avt{qb}_{i}") for i in range(CC)]
                for t in range(NKP):
                    et = ets.tile([P, 2, FB], FP8, tag="et")
                    for j in range(2):
                        kc = 2 * t + j
                        simt = psmm.tile([P, FB], F32, tag="mm")
                        for h in range(2):
                            nc.tensor.matmul(
                                simt,
                                lhsT=kp_sb[:, 2 * h:2 * h + 2, kc * P:(kc + 1) * P],
                                rhs=qp[:, 2 * h:2 * h + 2, :],
                                start=(h == 0), stop=(h == 1), perf_mode=DRM,
                            )
                        nc.scalar.activation(
                            out=et[:, j, :], in_=simt[:], func=AF.Exp, scale=SCALE,
                        )
                    nc.tensor.matmul(
                        den, lhsT=ones_col[:, :, :], rhs=et[:, :, :],
                        start=(t == 0), stop=(t == NKP - 1), perf_mode=DRM,
                    )
                    for c4 in range(CC):
                        nc.tensor.matmul(
                            avt[c4],
                            lhsT=vf_sb[:, 2 * t:2 * t + 2, c4 * P:(c4 + 1) * P],
                            rhs=et[:, :, :],
                            start=(t == 0), stop=(t == NKP - 1), perf_mode=DRM,
                        )

                # 1/den, broadcast on-chip (GPSIMD) to all partitions;
                # runs concurrently with the output-projection matmuls below.
                den_sb = xs.tile([1, FB], F32, tag="den_sb")
                nc.scalar.activation(out=den_sb, in_=den[0:1, :], func=AF.Copy)
                rec = xs.tile([1, FB], F32, tag="rec")
                nc.vector.reciprocal(out=rec, in_=den_sb[:])
                rbc = xs.tile([P, FB], F32, tag="rbc")
                nc.gpsimd.partition_broadcast(rbc[:, :], rec[:, :])

                # evict unnormalized attention output: xu = avt/64 = E^T v (bf16)
                xu = xus.tile([P, CC, FB], BF16, tag="xu")
                for c4 in range(CC):
                    nc.scalar.activation(
                        out=xu[:, c4, :], in_=avt[c4][:], func=AF.Copy, scale=1.0 / SV,
                    )

                for oc in range(CC):
                    po = psav.tile([P, FB], F32, tag="av", name=f"po{qb}_{oc}")
                    for cc in range(CC):
                        nc.tensor.matmul(
                            po, lhsT=wo_sb[:, cc, oc * P:(oc + 1) * P],
                            rhs=xu[:, cc, :],
                            start=(cc == 0), stop=(cc == CC - 1),
                        )
                    # out = po/den + bo
                    ot = outs.tile([P, FB], BF16, tag="ot")
                    nc.vector.tensor_mul(out=ot, in0=po[:], in1=rbc[:])
                    ob = outs.tile([P, FB], BF16, tag="ob")
                    nc.vector.tensor_scalar_add(out=ob, in0=ot[:], scalar1=bo_sb[:, oc:oc + 1])
                    nc.sync.dma_start(out=out_r[:, oc, qsl], in_=ob[:])

    nc.finalize()
    return nc


_NC_CACHE = {}


def _get_nc() -> bass.Bass:
    if "nc" not in _NC_CACHE:
        _NC_CACHE["nc"] = _build_bass()
    return _NC_CACHE["nc"]


def make_in_maps(q, Wq, bq, Wk, bk, Wv, bv, Wo, bo):
    f = np.float32
    f8 = ml_dtypes.float8_e4m3
    bf = ml_dtypes.bfloat16
    wq = (np.asarray(Wq, f).T * SW).astype(f8)
    wk_full = np.asarray(Wk, f).T * SW
    wk = wk_full.astype(f8)
    wkr = (wk_full - wk.astype(f)).astype(f8)
    wv = (np.asarray(Wv, f).T * SV).astype(f8)
    wo = np.asarray(Wo, f).T.astype(bf)

    def to_pwo(w):  # [C(c), C(o)] -> [p, cc, o] with c = cc*128 + p
        return w.reshape(CC, P, C).transpose(1, 0, 2)

    w1 = np.ascontiguousarray(
        np.concatenate([to_pwo(wv), to_pwo(wk), to_pwo(wkr)], axis=1))
    w2 = np.ascontiguousarray(to_pwo(wq))
    wo = np.ascontiguousarray(to_pwo(wo))
    bq = np.asarray(bq, f)
    bk = np.asarray(bk, f)
    bo = np.asarray(bo, f) + np.asarray(Wo, f) @ np.asarray(bv, f)
    in_maps = []
    for core in range(N_CORES):
        b, half = core // 2, core % 2
        x = np.asarray(q[b], f).reshape(C, S).astype(f8)
        # [C, S] -> [p, sb, cc, j]  (c = cc*128 + p, s = sb*512 + j)
        xa = np.ascontiguousarray(
            x.reshape(CC, P, NSB, FB).transpose(1, 2, 0, 3))
        xqh = x[:, half * QH:(half + 1) * QH]
        xq = np.ascontiguousarray(
            xqh.reshape(CC, P, NQB, FB).transpose(1, 2, 0, 3))
        x = xa
        in_maps.append({
            "x": np.ascontiguousarray(x), "xq": xq,
            "w1": w1, "w2": w2, "wo": wo,
            "bq": bq, "bk": bk, "bo": bo,
        })
    return in_maps


def gather_out(per_core_outs):
    out = np.zeros((B, C, S), np.float32)
    for core in range(N_CORES):
        b, half = core // 2, core % 2
        out[b, :, half * QH:(half + 1) * QH] = np.asarray(per_core_outs[core]).astype(np.float32)
    return out.reshape(B, C, H, W)


def kernel(q, Wq, bq, Wk, bk, Wv, bv, Wo, bo):
    nc = _get_nc()
    in_maps = make_in_maps(q, Wq, bq, Wk, bk, Wv, bv, Wo, bo)
    res = run_bass_kernel_spmd(nc, in_maps, core_ids=list(range(N_CORES)))
    return gather_out([res.results[i]["out"] for i in range(N_CORES)])


# revision 5
# speedup vs baseline: 1.0788x; 1.0741x over previous
"""Trainium2 Bass kernel for CrossAttention3D — fp8e4 DoubleRow version.

Math (per batch b, x = q[b] viewed (C, S)):
    qp = Wq x + bq ; kp = Wk x + bk ; vf = x^T Wv^T + bv
    simT = kp^T qp * C^-0.5 ; E = exp(simT) ; den = colsum(E)
    out = Wo (vf^T E) / den + bo

Q/K/V projections and attention matmuls run as fp8e4 DoubleRow
(contraction 256/instr, 2x bf16 PE throughput). Static scales keep every
fp8 operand in e4m3's good range (max 240, min normal 2^-6):
    wq8/wk8 = 256*W.T (wk also carries an fp8 residual term for
    accuracy), wv8 = 64*Wv.T, x8 = x (sigma~1); qp8/kp8 true-scale
    (sigma~0.45); et = exp(C^-0.5 * sim) in [0.2, 4.6]; vf8 = 64*v.
xu (= E^T v) is evicted in bf16 and the output projection runs in bf16
to keep the final-path quantization error small. out = po/den + bo.

Sharding: 8 cores = 4 batches x 2 query-halves. All inputs are
pre-arranged host-side into SBUF layout so DMA lines are contiguous.
Engines: PE matmuls (incl. ones-colsum den); ACT kp/qp/xu evict + exp;
DVE vf evict + final normalize; GPSIMD broadcasts 1/den on-chip;
normalization applied after the (linear) output projection so the
reciprocal chain overlaps the O-proj matmuls.
"""

import numpy as np
import ml_dtypes

import concourse.bass as bass
import concourse.bacc as bacc
import concourse.tile as tile
from concourse import mybir
from concourse.bass_utils import run_bass_kernel_spmd

AF = mybir.ActivationFunctionType
F32 = mybir.dt.float32
BF16 = mybir.dt.bfloat16
FP8 = mybir.dt.float8e4
DRM = mybir.MatmulPerfMode.DoubleRow

B, C, H, W = 4, 512, 64, 64
S = H * W            # 4096 tokens
P = 128              # partitions
CC = C // P          # 4 channel chunks
QH = S // 2          # 2048 queries per core
FB = 512             # free-dim block
NQB = QH // FB       # 4 query blocks per core
NSB = S // FB        # 8 token blocks
NKC = S // P         # 32 key chunks
NKP = NKC // 2       # 16 key chunk pairs
N_CORES = 8

SW = 256.0           # weight scale for Wq/Wk/Wo
SV = 64.0            # weight scale for Wv
SCALE = float(C) ** -0.5


def _build_bass() -> bass.Bass:
    nc = bacc.Bacc("TRN2", target_bir_lowering=False)

    # all inputs pre-arranged host-side into SBUF layout (contiguous
    # per-partition DMA lines): x/xq [p, blk, cc, 512]
    x_d = nc.dram_tensor("x", [P, NSB, CC, FB], FP8, kind="ExternalInput")
    xq_d = nc.dram_tensor("xq", [P, NQB, CC, FB], FP8, kind="ExternalInput")
    # packed fp8 weights [p, w, o]: w1 = [wv|wk|wkr], w2 = [wq|wqr]
    w1_d = nc.dram_tensor("w1", [P, 3 * CC, C], FP8, kind="ExternalInput")
    w2_d = nc.dram_tensor("w2", [P, CC, C], FP8, kind="ExternalInput")
    wo_d = nc.dram_tensor("wo", [P, CC, C], BF16, kind="ExternalInput")  # Wo.T
    bq_d = nc.dram_tensor("bq", [C], F32, kind="ExternalInput")
    bk_d = nc.dram_tensor("bk", [C], F32, kind="ExternalInput")
    bo_d = nc.dram_tensor("bo", [C], F32, kind="ExternalInput")    # bo + Wo@bv
    out_d = nc.dram_tensor("out", [C, QH], BF16, kind="ExternalOutput")

    out_r = out_d[:, :].rearrange("(oc p) s -> p oc s", p=P)

    def bcast_ap(ap_1d, parts):
        # [n] -> [parts, n] via 0-stride partition dim (DMA-only pattern)
        return bass.AP(
            tensor=ap_1d.tensor, offset=ap_1d.offset,
            ap=[[0, parts]] + [list(d) for d in ap_1d.ap],
        )

    with tile.TileContext(nc) as tc:
        with (
            tc.tile_pool(name="consts", bufs=1) as consts,
            tc.tile_pool(name="kv", bufs=1) as kv,
            tc.tile_pool(name="xs", bufs=3) as xs,
            tc.tile_pool(name="qps", bufs=2) as qps,
            tc.tile_pool(name="ets", bufs=4) as ets,
            tc.tile_pool(name="xus", bufs=3) as xus,
            tc.tile_pool(name="outs", bufs=4) as outs,
            tc.tile_pool(name="dscr", bufs=2, space="DRAM") as dscr,
            tc.tile_pool(name="psmm", bufs=3, space="PSUM") as psmm,
            tc.tile_pool(name="psav", bufs=4, space="PSUM") as psav,
            tc.tile_pool(name="psden", bufs=1, space="PSUM") as psden,
        ):
            # ---- constants; first x block + phase-1 weights lead so PE
            # ---- starts as early as possible, Wq/Wo deferred to phase 2 ----
            xt_first = xs.tile([P, CC, FB], FP8, tag="xt", name="xt_first")
            nc.sync.dma_start(out=xt_first, in_=x_d[:, 0, :, :])
            # weights on the scalar queue (parallel to sync queue): wv alone
            # first so the first V matmul starts ASAP, then wk+wkr
            w1_sb = consts.tile([P, 3 * CC, C], FP8, tag="w1")
            nc.scalar.dma_start(out=w1_sb[:, 0:CC, :], in_=w1_d[:, 0:CC, :])
            nc.scalar.dma_start(out=w1_sb[:, CC:3 * CC, :], in_=w1_d[:, CC:3 * CC, :])
            wv_sb = w1_sb[:, 0:CC, :]
            wk_sb = w1_sb[:, CC:2 * CC, :]
            wkr_sb = w1_sb[:, 2 * CC:3 * CC, :]
            bk_sb = consts.tile([P, CC], F32, tag="bk")
            nc.sync.dma_start(out=bk_sb, in_=bk_d[:].rearrange("(cc p) -> p cc", p=P))
            ones_col = consts.tile([P, 2, 16], FP8, tag="ones_col")
            nc.vector.memset(ones_col, 1.0)

            # ---- persistent K / V ----
            kp_sb = kv.tile([P, CC, S], FP8, tag="kp")
            vf_sb = kv.tile([P, NKC, C], FP8, tag="vf")

            # ---- phase 1: K and V projections over the full sequence ----
            for sb in range(NSB):
                if sb == 0:
                    xt = xt_first
                else:
                    xt = xs.tile([P, CC, FB], FP8, tag="xt")
                    nc.sync.dma_start(out=xt, in_=x_d[:, sb, :, :])
                for i4 in range(CC):
                    sc = sb * CC + i4
                    psv = psav.tile([P, FB], F32, tag="av")
                    for h in range(2):
                        nc.tensor.matmul(
                            psv, lhsT=xt[:, 2 * h:2 * h + 2, i4 * P:(i4 + 1) * P],
                            rhs=wv_sb[:, 2 * h:2 * h + 2, :],
                            start=(h == 0), stop=(h == 1), perf_mode=DRM,
                        )
                    # vf = 64*v; bv is folded into bo via bo' = bo + Wo@bv
                    nc.vector.tensor_scalar_mul(out=vf_sb[:, sc, :], in0=psv[:], scalar1=1.0)
                for oc in range(CC):
                    ps = psmm.tile([P, FB], F32, tag="mm")
                    for wt, h in ((w, h) for w in (wk_sb, wkr_sb) for h in range(2)):
                        nc.tensor.matmul(
                            ps, lhsT=wt[:, 2 * h:2 * h + 2, oc * P:(oc + 1) * P],
                            rhs=xt[:, 2 * h:2 * h + 2, :],
                            start=(wt is wk_sb and h == 0),
                            stop=(wt is wkr_sb and h == 1), perf_mode=DRM,
                        )
                    nc.scalar.activation(
                        out=kp_sb[:, oc, sb * FB:(sb + 1) * FB], in_=ps[:],
                        func=AF.Identity, bias=bk_sb[:, oc:oc + 1], scale=1.0 / SW,
                    )

            # ---- deferred constants for phase 2 ----
            w2_sb = consts.tile([P, CC, C], FP8, tag="w2")
            nc.scalar.dma_start(out=w2_sb, in_=w2_d[:, :, :])
            wq_sb = w2_sb[:, 0:CC, :]
            wo_sb = consts.tile([P, CC, C], BF16, tag="wo")
            nc.scalar.dma_start(out=wo_sb, in_=wo_d[:, :, :])
            bq_sb = consts.tile([P, CC], F32, tag="bq")
            nc.sync.dma_start(out=bq_sb, in_=bq_d[:].rearrange("(cc p) -> p cc", p=P))
            bo_sb = consts.tile([P, CC], F32, tag="bo")
            nc.sync.dma_start(out=bo_sb, in_=bo_d[:].rearrange("(cc p) -> p cc", p=P))

            # ---- phase 2: per query-block fused attention ----
            for qb in range(NQB):
                qsl = slice(qb * FB, (qb + 1) * FB)

                # Q projection
                xqt = xs.tile([P, CC, FB], FP8, tag="xt")
                nc.sync.dma_start(out=xqt, in_=xq_d[:, qb, :, :])
                qp = qps.tile([P, CC, FB], FP8, tag="qp")
                for oc in range(CC):
                    ps = psmm.tile([P, FB], F32, tag="mm")
                    for h in range(2):
                        nc.tensor.matmul(
                            ps, lhsT=wq_sb[:, 2 * h:2 * h + 2, oc * P:(oc + 1) * P],
                            rhs=xqt[:, 2 * h:2 * h + 2, :],
                            start=(h == 0), stop=(h == 1), perf_mode=DRM,
                        )
                    nc.scalar.activation(
                        out=qp[:, oc, :], in_=ps[:],
                        func=AF.Identity, bias=bq_sb[:, oc:oc + 1], scale=1.0 / SW,
                    )

                den = psden.tile([16, FB], F32, tag="den")
                avt = [psav.tile([P, FB], F32, tag="av", name=f"avt{qb}_{i}") for i in range(CC)]
                for t in range(NKP):
                    et = ets.tile([P, 2, FB], FP8, tag="et")
                    for j in range(2):
                        kc = 2 * t + j
                        simt = psmm.tile([P, FB], F32, tag="mm")
                        for h in range(2):
                            nc.tensor.matmul(
                                simt,
                                lhsT=kp_sb[:, 2 * h:2 * h + 2, kc * P:(kc + 1) * P],
                                rhs=qp[:, 2 * h:2 * h + 2, :],
                                start=(h == 0), stop=(h == 1), perf_mode=DRM,
                            )
                        nc.scalar.activation(
                            out=et[:, j, :], in_=simt[:], func=AF.Exp, scale=SCALE,
                        )
                    nc.tensor.matmul(
                        den, lhsT=ones_col[:, :, :], rhs=et[:, :, :],
                        start=(t == 0), stop=(t == NKP - 1), perf_mode=DRM,
                    )
                    for c4 in range(CC):
                        nc.tensor.matmul(
                            avt[c4],
                            lhsT=vf_sb[:, 2 * t:2 * t + 2, c4 * P:(c4 + 1) * P],
                            rhs=et[:, :, :],
                            start=(t == 0), stop=(t == NKP - 1), perf_mode=DRM,
                        )

                # 1/den, broadcast on-chip (GPSIMD) to all partitions;
                # runs concurrently with the output-projection matmuls below.
                den_sb = xs.tile([1, FB], F32, tag="den_sb")
                nc.scalar.activation(out=den_sb, in_=den[0:1, :], func=AF.Copy)
                rec = xs.tile([1, FB], F32, tag="rec")
                nc.vector.reciprocal(out=rec, in_=den_sb[:])
                rbc = xs.tile([P, FB], F32, tag="rbc")
                nc.gpsimd.partition_broadcast(rbc[:, :], rec[:, :])

                # evict unnormalized attention output: xu = avt/64 = E^T v (bf16)
                xu = xus.tile([P, CC, FB], BF16, tag="xu")
                for c4 in range(CC):
                    nc.scalar.activation(
                        out=xu[:, c4, :], in_=avt[c4][:], func=AF.Copy, scale=1.0 / SV,
                    )

                for oc in range(CC):
                    po = psav.tile([P, FB], F32, tag="av", name=f"po{qb}_{oc}")
                    for cc in range(CC):
                        nc.tensor.matmul(
                            po, lhsT=wo_sb[:, cc, oc * P:(oc + 1) * P],
                            rhs=xu[:, cc, :],
                            start=(cc == 0), stop=(cc == CC - 1),
                        )
                    # out = po/den + bo
                    ot = outs.tile([P, FB], BF16, tag="ot")
                    nc.vector.tensor_mul(out=ot, in0=po[:], in1=rbc[:])
                    ob = outs.tile([P, FB], BF16, tag="ob")
                    nc.vector.tensor_scalar_add(out=ob, in0=ot[:], scalar1=bo_sb[:, oc:oc + 1])
                    nc.sync.dma_start(out=out_r[:, oc, qsl], in_=ob[:])

    nc.finalize()
    return nc


_NC_CACHE = {}


def _get_nc() -> bass.Bass:
    if "nc" not in _NC_CACHE:
        _NC_CACHE["nc"] = _build_bass()
    return _NC_CACHE["nc"]


def make_in_maps(q, Wq, bq, Wk, bk, Wv, bv, Wo, bo):
    f = np.float32
    f8 = ml_dtypes.float8_e4m3
    bf = ml_dtypes.bfloat16
    wq = (np.asarray(Wq, f).T * SW).astype(f8)
    wk_full = np.asarray(Wk, f).T * SW
    wk = wk_full.astype(f8)
    wkr = (wk_full - wk.astype(f)).astype(f8)
    wv = (np.asarray(Wv, f).T * SV).astype(f8)
    wo = np.asarray(Wo, f).T.astype(bf)

    def to_pwo(w):  # [C(c), C(o)] -> [p, cc, o] with c = cc*128 + p
        return w.reshape(CC, P, C).transpose(1, 0, 2)

    w1 = np.ascontiguousarray(
        np.concatenate([to_pwo(wv), to_pwo(wk), to_pwo(wkr)], axis=1))
    w2 = np.ascontiguousarray(to_pwo(wq))
    wo = np.ascontiguousarray(to_pwo(wo))
    bq = np.asarray(bq, f)
    bk = np.asarray(bk, f)
    bo = np.asarray(bo, f) + np.asarray(Wo, f) @ np.asarray(bv, f)
    in_maps = []
    for core in range(N_CORES):
        b, half = core // 2, core % 2
        x = np.asarray(q[b], f).reshape(C, S).astype(f8)
        # [C, S] -> [p, sb, cc, j]  (c = cc*128 + p, s = sb*512 + j)
        xa = np.ascontiguousarray(
            x.reshape(CC, P, NSB, FB).transpose(1, 2, 0, 3))
        xqh = x[:, half * QH:(half + 1) * QH]
        xq = np.ascontiguousarray(
            xqh.reshape(CC, P, NQB, FB).transpose(1, 2, 0, 3))
        x = xa
        in_maps.append({
            "x": np.ascontiguousarray(x), "xq": xq,
            "w1": w1, "w2": w2, "wo": wo,
            "bq": bq, "bk": bk, "bo": bo,
        })
    return in_maps


def gather_out(per_core_outs):
    out = np.zeros((B, C, S), np.float32)
    for core in range(N_CORES):
        b, half = core // 2, core % 2
        out[b, :, half * QH:(half + 1) * QH] = np.asarray(per_core_outs[core]).astype(np.float32)
    return out.reshape(B, C, H, W)


def kernel(q, Wq, bq, Wk, bk, Wv, bv, Wo, bo):
    nc = _get_nc()
    in_maps = make_in_maps(q, Wq, bq, Wk, bk, Wv, bv, Wo, bo)
    res = run_bass_kernel_spmd(nc, in_maps, core_ids=list(range(N_CORES)))
    return gather_out([res.results[i]["out"] for i in range(N_CORES)])
